# revision 14
# baseline (speedup 1.0000x reference)
"""AMIPRouter Trainium2 kernel (8 NeuronCores, SPMD, no collectives).

Math restructure (exactly equivalent to the reference):
  eo[t,k,:]   = gelu(h[t] @ W1_k + b1_k) @ W2_k + b2_k
  win[s,k,:]  = sum_{t in window(s), t unmasked} eo[t,k,:]
  out[s]      = LN( sum_k w[s,k] * win[s,k,:] / cnt[s] )  at s masked & cnt>0

W2 is linear, so the windowed neighbor-sum commutes with it:
  win[s,k,:] = (sum_{t in win(s)} ghid[t,k,:]) @ W2_k + cnt[s] * b2_k
with ghid = gelu(layer1) over *unmasked* tokens only. The positional windowed
sum becomes a matmul against a host-built 0/1 selection matrix Wsel[j, m]
(j: unmasked tokens in the shard's halo range, m: masked+valid outputs); all
mask-dependent gather/scatter is host-side sharding prep. Per core:
  L1:   ghid[j,f;k] = gelu(hgT.T @ W1_k + b1_k)      (unmasked tokens only)
  WIN:  winT[f,m;k] = ghid_k.T @ Wsel
  A     = winT * broadcast(w[m,k] / cnt[m])          (w = on-device softmax)
  L2:   mixed[m,:]  = A.T @ W2_flat + w.T @ b2       (masked outputs only)
  LN + host scatter back into [B,S,D] (invalid rows stay zero).

Sharding: data-parallel over (batch, seq quarter) -> 8 shards of 512
positions; window radius r<=8 handled by host-side halo in the gather.
All device inputs are laid out partition-major on the host so every DMA is
a fully linear per-partition transfer.
"""

import numpy as np
import ml_dtypes

BF16 = ml_dtypes.bfloat16

_B, _S, _D, _K, _F = 2, 2048, 2048, 8, 512
_NCORES = 8
_QS = _S // 4  # 512 output positions per shard

_GRAPH_CACHE = {}


def _ceil_mult(x, m):
    return max(m, ((x + m - 1) // m) * m)


def _build_graph(NU, SM, SMA):
    """Build + compile the per-core Bass graph for padded sizes (NU, SM)."""
    import concourse.mybir as mybir
    from concourse import bacc
    from concourse.tile import TileContext
    from concourse.masks import make_identity
    from contextlib import ExitStack

    D, K, F = _D, _K, _F
    DC = D // 128          # 16 contract chunks of d
    FM = F // 128          # 4 f-chunks per expert
    KF = K * F // 128      # 32 contract chunks of layer 2
    DN = D // 512          # 4 output d chunks
    JC = NU // 128
    SC = SM // 128
    f32 = mybir.dt.float32
    bf16 = mybir.dt.bfloat16
    AX = mybir.AxisListType.X
    AF = mybir.ActivationFunctionType
    ALU = mybir.AluOpType

    nc = bacc.Bacc("TRN2", target_bir_lowering=False, debug=False, num_devices=_NCORES)

    # all big inputs are pre-laid-out partition-major: [128, ...]
    hgT_e = nc.declare_dram_parameter("hgT", [128, DC, NU], bf16, isOutput=False)
    hmT_e = nc.declare_dram_parameter("hmT", [128, DC, SM], bf16, isOutput=False)
    wsel_e = nc.declare_dram_parameter("wsel", [128, JC, SM], bf16, isOutput=False)
    invc_e = nc.declare_dram_parameter("invc", [SC, 128, 1], f32, isOutput=False)
    w1_e = nc.declare_dram_parameter("w1", [K, 128, DC, F], bf16, isOutput=False)
    w2_e = nc.declare_dram_parameter("w2", [DC, 128, KF, 128], bf16, isOutput=False)
    b1_e = nc.declare_dram_parameter("b1", [1, K, F], bf16, isOutput=False)
    b2_e = nc.declare_dram_parameter("b2", [K, D], bf16, isOutput=False)
    wr_e = nc.declare_dram_parameter("wroute", [128, DC, K], bf16, isOutput=False)
    br_e = nc.declare_dram_parameter("broute", [1, K], bf16, isOutput=False)
    out_e = nc.declare_dram_parameter("out", [SM, D], bf16, isOutput=True)

    with TileContext(nc) as tc, ExitStack() as ctx:
        const = ctx.enter_context(tc.tile_pool(name="const", bufs=1))
        A_pool = ctx.enter_context(tc.tile_pool(name="Apool", bufs=1))
        w2p = ctx.enter_context(tc.tile_pool(name="w2p", bufs=2))
        stage_p = ctx.enter_context(tc.tile_pool(name="stagep", bufs=1))
        w1p = ctx.enter_context(tc.tile_pool(name="w1p", bufs=2))
        ghp = ctx.enter_context(tc.tile_pool(name="ghp", bufs=2))

        # ---- DMA issuance, in priority order ----
        # 1) what layer-1 of expert 0 needs, interleaved by d-group
        w1t = {}
        w1t[0] = w1p.tile([128, DC, F], bf16, name="w1t_0", tag="w1t")
        hgT_sb = const.tile([128, DC, NU], bf16, name="hgT_sb")
        for g in range(4):
            sl = slice(g * 4, (g + 1) * 4)
            nc.sync.dma_start(out=w1t[0][:, sl, :], in_=w1_e[0][:, sl, :])
            nc.sync.dma_start(out=hgT_sb[:, sl, :], in_=hgT_e[:][:, sl, :])
        # 2) routing inputs + window selection
        hmT_sb = const.tile([128, DC, SM], bf16, name="hmT_sb")
        for g in range(4):
            sl = slice(g * 4, (g + 1) * 4)
            nc.gpsimd.dma_start(out=hmT_sb[:, sl, :], in_=hmT_e[:][:, sl, :])
        wsel_sb = const.tile([128, JC, SM], bf16, name="wsel_sb")
        nc.gpsimd.dma_start(out=wsel_sb, in_=wsel_e[:])
        wr_sb = const.tile([128, DC, K], bf16, name="wr_sb")
        nc.gpsimd.dma_start(out=wr_sb, in_=wr_e[:])
        br_sb = const.tile([1, K], bf16, name="br_sb")
        nc.gpsimd.dma_start(out=br_sb, in_=br_e[:])
        b1_sb = const.tile([1, K, F], bf16, name="b1_sb")
        nc.gpsimd.dma_start(out=b1_sb, in_=b1_e[:])
        b2_sb = const.tile([K, D], bf16, name="b2_sb")
        nc.gpsimd.dma_start(out=b2_sb, in_=b2_e[:])
        invc_sb = const.tile([128, SC], f32, name="invc_sb")
        for sc in range(SC):
            nc.gpsimd.dma_start(out=invc_sb[:, sc : sc + 1], in_=invc_e[sc])
        w2c = {}

        # ---- constants ----
        ones_bf = const.tile([1, 128], bf16, name="ones_bf")
        nc.vector.memset(ones_bf, 1.0)
        ones_f32 = const.tile([1, 128], f32, name="ones_f32")
        nc.vector.memset(ones_f32, 1.0)
        ident = const.tile([128, 128], f32, name="ident")
        make_identity(nc, ident)
        ident_bf = const.tile([128, 128], bf16, name="ident_bf")
        make_identity(nc, ident_bf)
        eps_t = const.tile([128, 1], f32, name="eps_t")
        nc.vector.memset(eps_t, 1e-5)

        wT_sb = const.tile([K, SM], bf16, name="wT_sb")
        wiT_sb = const.tile([K, SM], f32, name="wiT_sb")
        wiT_row = const.tile([1, K, SM], bf16, name="wiT_row")
        wbc = const.tile([128, K, SMA], bf16, name="wbc")
        A_tiles = {}

        with (
            tc.tile_pool(name="ps1", bufs=3, space="PSUM") as ps1,
            tc.tile_pool(name="psw", bufs=3, space="PSUM") as psw,
        ):

            def layer1(k):
                ghid_k = []
                for jc in range(JC):
                    pg = ps1.tile([128, F], f32, name=f"pg_{k}_{jc}", tag="pg")
                    for dc in range(DC):
                        nc.tensor.matmul(
                            pg,
                            lhsT=hgT_sb[:, dc, jc * 128 : (jc + 1) * 128],
                            rhs=w1t[k][:, dc, :],
                            start=(dc == 0),
                            stop=False,
                        )
                    nc.tensor.matmul(
                        pg, lhsT=ones_bf[0:1, :], rhs=b1_sb[0:1, k, :],
                        start=False, stop=True,
                    )
                    gt = ghp.tile([128, F], bf16, name=f"gh_{k}_{jc}",
                                  tag=f"gh_{jc}")
                    nc.scalar.activation(gt, pg, AF.Gelu)
                    ghid_k.append(gt)
                return ghid_k

            def window(k, ghid_k):
                for fm in range(FM):
                    At = A_pool.tile([128, SMA], bf16, name=f"A_{k}_{fm}",
                                     tag=f"A_{k}_{fm}")
                    A_tiles[(k, fm)] = At
                    for n0 in range(0, SMA, 512):
                        n1 = min(SMA, n0 + 512)
                        pw = psw.tile([128, n1 - n0], f32,
                                      name=f"pw_{k}_{fm}_{n0}", tag="pw")
                        for jc in range(JC):
                            nc.tensor.matmul(
                                pw,
                                lhsT=ghid_k[jc][:, fm * 128 : (fm + 1) * 128],
                                rhs=wsel_sb[:, jc, n0:n1],
                                start=(jc == 0),
                                stop=(jc == JC - 1),
                            )
                        nc.vector.tensor_mul(At[:, n0:n1], pw, wbc[:, k, n0:n1])

            # ---- routing softmax ----
            with (
                tc.tile_pool(name="psum_r", bufs=1, space="PSUM") as psum_r,
                tc.tile_pool(name="rtmp", bufs=3) as rtmp,
            ):
                for sc in range(SC):
                    pr = psum_r.tile([128, K], f32, name=f"pr_{sc}", tag="pr")
                    for dc in range(DC):
                        nc.tensor.matmul(
                            pr,
                            lhsT=hmT_sb[:, dc, sc * 128 : (sc + 1) * 128],
                            rhs=wr_sb[:, dc, :],
                            start=(dc == 0),
                            stop=False,
                        )
                    nc.tensor.matmul(
                        pr, lhsT=ones_bf[0:1, :], rhs=br_sb[0:1, :],
                        start=False, stop=True,
                    )
                    mx = rtmp.tile([128, 1], f32, name=f"mx_{sc}", tag="mx")
                    nc.vector.reduce_max(mx, pr, axis=AX)
                    negmx = rtmp.tile([128, 1], f32, name=f"negmx_{sc}", tag="negmx")
                    nc.scalar.mul(negmx, mx, -1.0)
                    ex = rtmp.tile([128, K], f32, name=f"ex_{sc}", tag="ex")
                    nc.scalar.activation(ex, pr, AF.Exp, bias=negmx)
                    sm_ = rtmp.tile([128, 1], f32, name=f"sm_{sc}", tag="sm")
                    nc.vector.reduce_sum(sm_, ex, axis=AX)
                    rs = rtmp.tile([128, 1], f32, name=f"rs_{sc}", tag="rs")
                    nc.vector.reciprocal(rs, sm_)
                    w_t = rtmp.tile([128, K], f32, name=f"w_t_{sc}", tag="w_t")
                    nc.scalar.mul(w_t, ex, rs)
                    wi_t = rtmp.tile([128, K], f32, name=f"wi_t_{sc}", tag="wi_t")
                    nc.vector.tensor_scalar_mul(wi_t, w_t, invc_sb[:, sc : sc + 1])
                    wtp = psum_r.tile([K, 128], f32, name=f"wtp_{sc}", tag="wt",
                                      bufs=1)
                    nc.tensor.transpose(wtp, w_t, ident)
                    nc.vector.tensor_copy(wT_sb[:, sc * 128 : (sc + 1) * 128], wtp)
                    wip = psum_r.tile([K, 128], f32, name=f"wip_{sc}", tag="wt",
                                      bufs=1)
                    nc.tensor.transpose(wip, wi_t, ident)
                    nc.vector.tensor_copy(wiT_sb[:, sc * 128 : (sc + 1) * 128], wip)

            # broadcast w/cnt rows across partitions via outer product
            # (PE operands need base partition 0 -> DMA each row to partition 0)
            with tc.tile_pool(name="psum_b", bufs=1, space="PSUM") as psum_b:
                for k in range(K):
                    # gpsimd DMA casts f32 -> bf16 while moving to partition 0
                    nc.gpsimd.dma_start(
                        out=wiT_row[0:1, k, :], in_=wiT_sb[k : k + 1, :]
                    )
                for k in range(K):
                    for n0 in range(0, SMA, 512):
                        n1 = min(SMA, n0 + 512)
                        pb = psum_b.tile([128, n1 - n0], f32, name=f"pb_{k}_{n0}",
                                         tag="pb")
                        nc.tensor.matmul(
                            pb, lhsT=ones_bf[0:1, :],
                            rhs=wiT_row[0:1, k, n0:n1],
                            start=True, stop=True,
                        )
                        nc.scalar.copy(wbc[:, k, n0:n1], pb)

            ghid_0 = layer1(0)
            # prefetch expert-1 weights, then finish expert 0
            w1t[1] = w1p.tile([128, DC, F], bf16, name="w1t_1", tag="w1t")
            for g in range(4):
                sl = slice(g * 4, (g + 1) * 4)
                nc.sync.dma_start(out=w1t[1][:, sl, :], in_=w1_e[1][:, sl, :])
            window(0, ghid_0)

            for k in range(1, K):
                if k + 1 < K:
                    w1t[k + 1] = w1p.tile([128, DC, F], bf16,
                                          name=f"w1t_{k + 1}", tag="w1t")
                    for g in range(4):
                        sl = slice(g * 4, (g + 1) * 4)
                        nc.sync.dma_start(
                            out=w1t[k + 1][:, sl, :], in_=w1_e[k + 1][:, sl, :]
                        )
                ghid_k = layer1(k)
                window(k, ghid_k)

        # ---- Phase C: transposed layer-2 (out = mixed^T chunks), then
        # PE transpose back to [sm, d], LayerNorm, bf16 output ----
        stage = stage_p.tile([128, SC, D], bf16, name="stage")
        with (
            tc.tile_pool(name="w2sp", bufs=4) as w2sp,
            tc.tile_pool(name="mtp", bufs=3) as mtp,
            tc.tile_pool(name="ps2", bufs=3, space="PSUM") as ps2,
            tc.tile_pool(name="ptp", bufs=2, space="PSUM") as ptp,
            tc.tile_pool(name="lnt", bufs=2) as lnt,
            tc.tile_pool(name="ost", bufs=4) as ost,
        ):
            w2s = {}

            def fetch_w2(dc):
                w2s[dc] = w2sp.tile([128, KF, 128], bf16, name=f"w2s_{dc}",
                                    tag="w2s")
                eng = nc.sync if dc % 2 == 0 else nc.gpsimd
                eng.dma_start(out=w2s[dc], in_=w2_e[dc])

            for dc in range(3):
                fetch_w2(dc)
            for dc in range(DC):
                if dc + 3 < DC:
                    fetch_w2(dc + 3)
                p2 = ps2.tile([128, SMA], f32, name=f"p2_{dc}", tag="p2")
                for c in range(KF):
                    nc.tensor.matmul(
                        p2,
                        lhsT=w2s[dc][:, c, :],
                        rhs=A_tiles[(c // FM, c % FM)],
                        start=(c == 0),
                        stop=False,
                    )
                nc.tensor.matmul(
                    p2,
                    lhsT=b2_sb[:, dc * 128 : (dc + 1) * 128],
                    rhs=wT_sb[:, 0:SMA],
                    start=False,
                    stop=True,
                )
                mt = mtp.tile([128, SMA], bf16, name=f"mt_{dc}", tag="mt")
                nc.scalar.copy(mt, p2)
                for sc in range(SC):
                    m0 = sc * 128
                    w = min(SMA, m0 + 128) - m0
                    if w <= 0:
                        continue
                    pt = ptp.tile([128, 128], bf16, name=f"pt_{dc}_{sc}", tag="pt")
                    nc.tensor.transpose(pt[0:w, :], mt[:, m0 : m0 + w], ident_bf)
                    nc.scalar.copy(stage[0:w, sc, dc * 128 : (dc + 1) * 128],
                                   pt[0:w, :])
            for sc in range(SC):
                stats = lnt.tile([128, DN, 6], f32, name=f"stats_{sc}",
                                 tag="stats")
                for d2 in range(DN):
                    nc.vector.bn_stats(
                        stats[:, d2, :],
                        stage[:, sc, d2 * 512 : (d2 + 1) * 512],
                    )
                mv = lnt.tile([128, 2], f32, name=f"mv_{sc}", tag="mv")
                nc.vector.bn_aggr(mv, stats)
                negmean = lnt.tile([128, 1], f32, name=f"negmean_{sc}", tag="nm")
                nc.scalar.mul(negmean, mv[:, 0:1], -1.0)
                sd = lnt.tile([128, 1], f32, name=f"sd_{sc}", tag="sd")
                nc.scalar.activation(sd, mv[:, 1:2], AF.Sqrt, bias=eps_t)
                rstd = lnt.tile([128, 1], f32, name=f"rstd_{sc}", tag="rstd")
                nc.vector.reciprocal(rstd, sd)
                for d2 in range(DN):
                    ot = ost.tile([128, 512], bf16, name=f"ot_{sc}_{d2}", tag="ot")
                    nc.vector.tensor_scalar(
                        ot,
                        stage[:, sc, d2 * 512 : (d2 + 1) * 512],
                        scalar1=negmean,
                        scalar2=rstd,
                        op0=ALU.add,
                        op1=ALU.mult,
                    )
                    dma_eng = nc.sync if d2 % 2 == 0 else nc.gpsimd
                    dma_eng.dma_start(
                        out=out_e[sc * 128 : (sc + 1) * 128,
                                  d2 * 512 : (d2 + 1) * 512],
                        in_=ot,
                    )

    nc.compile()
    return nc


def kernel(h_L, masked, W_route, b_route, W1, b1, W2, b2, range_r):
    R = int(range_r)
    h_L = np.asarray(h_L, dtype=np.float32)
    masked = np.asarray(masked).astype(bool)
    B, S, D = h_L.shape
    K = W_route.shape[1]
    DC = D // 128

    unm = (~masked).astype(np.float64)
    cs = np.concatenate([np.zeros((B, 1)), np.cumsum(unm, axis=1)], axis=1)
    idx = np.arange(S)
    hi = np.clip(idx + R, 0, S - 1) + 1
    lo = np.clip(idx - R, 0, S)
    cnt = cs[:, hi] - cs[:, lo] - unm
    valid = masked & (cnt > 0)

    # balance shard boundaries so the max masked-valid count per shard is
    # minimized (it sets the padded matmul sizes for every core)
    shards = []
    for b in range(B):
        cv = np.cumsum(valid[b].astype(np.int64))
        tot = int(cv[-1])
        bounds = [0]
        for q in range(1, 4):
            bounds.append(int(np.searchsorted(cv, q * tot / 4.0)))
        bounds.append(S)
        for q in range(4):
            p0, p1 = bounds[q], bounds[q + 1]
            h0, h1 = max(0, p0 - R), min(S, p1 + R)
            upos = np.nonzero(unm[b, h0:h1] > 0)[0] + h0
            mpos = np.nonzero(valid[b, p0:p1])[0] + p0
            shards.append((b, upos, mpos))

    NU = _ceil_mult(max(len(u) for _, u, _ in shards), 128)
    SMA = _ceil_mult(max(len(m) for _, _, m in shards), 8)
    SM = _ceil_mult(SMA, 128)
    SC = SM // 128
    JC = NU // 128
    KF = K * _F // 128
    DN = D // 512

    # shared weight arrays, pre-laid-out partition-major for linear DMA
    w1b = np.ascontiguousarray(
        W1.astype(BF16).reshape(K, DC, 128, _F).transpose(0, 2, 1, 3)
    )  # [K, 128, DC, F]
    w2b = np.ascontiguousarray(
        np.asarray(W2)
        .reshape(KF, 128, DC, 128)
        .transpose(2, 1, 0, 3)
        .astype(BF16)
    )  # [DC, 128, KF, 128]
    b1b = np.ascontiguousarray(b1.astype(BF16).reshape(1, K, _F))
    b2b = np.ascontiguousarray(b2.astype(BF16))
    wrb = np.ascontiguousarray(
        W_route.astype(BF16).reshape(DC, 128, K).transpose(1, 0, 2)
    )  # [128, DC, K]
    brb = np.ascontiguousarray(np.asarray(b_route).reshape(1, K).astype(BF16))

    in_maps = []
    for b, upos, mpos in shards:
        nu, sm = len(upos), len(mpos)
        hgT = np.zeros((D, NU), dtype=BF16)
        hgT[:, :nu] = h_L[b, upos, :].T.astype(BF16)
        hmT = np.zeros((D, SM), dtype=BF16)
        hmT[:, :sm] = h_L[b, mpos, :].T.astype(BF16)
        wsel = np.zeros((NU, SM), dtype=BF16)
        if nu and sm:
            wsel[:nu, :sm] = (
                np.abs(upos[:, None] - mpos[None, :]) <= R
            ).astype(BF16)
        invc = np.zeros((SM, 1), dtype=np.float32)
        invc[:sm, 0] = (1.0 / cnt[b, mpos]).astype(np.float32)
        in_maps.append(
            {
                # partition-major relayouts
                "hgT": np.ascontiguousarray(
                    hgT.reshape(DC, 128, NU).transpose(1, 0, 2)
                ),
                "hmT": np.ascontiguousarray(
                    hmT.reshape(DC, 128, SM).transpose(1, 0, 2)
                ),
                "wsel": np.ascontiguousarray(
                    wsel.reshape(JC, 128, SM).transpose(1, 0, 2)
                ),
                "invc": invc.reshape(SC, 128, 1),
                "w1": w1b,
                "w2": w2b,
                "b1": b1b,
                "b2": b2b,
                "wroute": wrb,
                "broute": brb,
            }
        )

    key = (NU, SM, SMA)
    if key not in _GRAPH_CACHE:
        _GRAPH_CACHE[key] = _build_graph(NU, SM, SMA)
    nc = _GRAPH_CACHE[key]

    from concourse.bass_utils import run_bass_kernel_spmd

    res = run_bass_kernel_spmd(nc, in_maps, core_ids=list(range(_NCORES)))

    out = np.zeros((B, S, D), dtype=np.float32)
    for core, (b, _, mpos) in enumerate(shards):
        if len(mpos):
            out[b, mpos, :] = res.results[core]["out"][: len(mpos)].astype(
                np.float32
            )
    return out
